# revision 18
# baseline (speedup 1.0000x reference)
"""Multi-head attention (B=2, N=4096, D=768, H=12) on 8 Trainium2 NeuronCores.

Sharding: core c handles batch b = c//4 and heads [3g, 3g+1, 3g+2] with
g = c%4 (data parallel on B, head parallel on H). Each core computes its
heads' Q/K/V from x[b], runs softmax attention, and produces the partial
output projection for its head block; the host sums the 4 partials per
batch (row-parallel unshard) and adds b_proj.

v2 changes vs the 674us baseline (which was exp/ACT-throughput bound at
~1205ns per key-chunk with the PE at ~905ns):
  - x^T is pre-transposed and pre-cast to bf16 on the host, removing the
    in-kernel fp32 x load + DVE cast + xbar transpose (~40us of startup
    latency and DMA/DVE pressure).
  - softmax exp alternates between the ACT engine (native Exp) and the
    DVE (Schraudolph bit-trick: p = bitcast_bf16(int16(s*c1 + c2)),
    one tensor_scalar op), halving the softmax-stage cost so the
    attention phase runs at the PE's ~905ns/chunk cadence.
  - softmax denominator reciprocal via reciprocal_approx_fast on a
    [3, 512] batched tile (the baseline's [1,512] nc.vector.reciprocal
    ran single-lane at 8 cyc/elem: 95us -> ~7us).
  - QKV projection drops the duplicated-h2 matmul group (5 groups
    instead of 6); the h2 Q/K partition-duplicates for the even/odd
    score pairing are made with two small SBUF->SBUF DMAs per chunk.
  - O_h2 is stored zero-padded to 128 partitions so the projection
    matmul runs with a 128-row stationary (FWL-eligible).
  - proj PSUM->SBUF copies moved from DVE to the scalar engine.
"""

import numpy as np
import ml_dtypes
from contextlib import ExitStack

import concourse.bass as bass
from concourse import bacc
import concourse.tile as tile
import concourse.mybir as mybir
from concourse.bass_utils import run_bass_kernel_spmd

F32 = mybir.dt.float32
BF16 = mybir.dt.bfloat16
I16 = mybir.dt.int16
AF = mybir.ActivationFunctionType
ALU = mybir.AluOpType

B, N, D, H, HD = 2, 4096, 768, 12, 64
SCALE = HD ** -0.5
NC = 8
NCHUNK = N // 128          # 32 key chunks of 128
NQT = N // 512             # 8 query tiles of 512
NSC = N // 512             # 8 seq chunks of 512 (QKV stage)
KCH = D // 128             # 6 contraction chunks

# Schraudolph exp for the DVE half of the softmax:
#   p = bitcast_bf16(int16(s * EXP_C1 + EXP_C2)) ~= exp(s * SCALE)
# sigma = -5.25 balances the max relative error (~3.3%) for either
# truncating or round-to-nearest float->int conversion.
EXP_C1 = float(SCALE * np.log2(np.e) * 128.0)
EXP_C2 = 16256.0 - 5.25


def build_program():
    nc = bacc.Bacc("TRN2", target_bir_lowering=False, debug=False)

    # xt[j, p, k, m] = x[512j + m, 128k + p] in bf16 (host pre-transposed)
    xt = nc.dram_tensor("xt", [NSC, 128, KCH, 512], BF16, kind="ExternalInput").ap()
    wg = nc.dram_tensor("wg", [5, D, 128], BF16, kind="ExternalInput").ap()
    bias = nc.dram_tensor("bias", [128, 5], F32, kind="ExternalInput").ap()
    wpp = nc.dram_tensor("wpp", [128, D], BF16, kind="ExternalInput").ap()
    wp2z = nc.dram_tensor("wp2z", [128, D], BF16, kind="ExternalInput").ap()
    y = nc.dram_tensor("y", [N, D], F32, kind="ExternalOutput").ap()

    with tile.TileContext(nc) as tc, ExitStack() as octx:
        const = octx.enter_context(tc.tile_pool(name="const", bufs=1))
        qkpool = octx.enter_context(tc.tile_pool(name="qk", bufs=1))
        vpool = octx.enter_context(tc.tile_pool(name="vaug", bufs=1))
        opool_sb = octx.enter_context(tc.tile_pool(name="onorm", bufs=1))

        bias_sb = const.tile([128, 5], F32)
        wpp_sb = const.tile([128, D], BF16)
        wp2z_sb = const.tile([128, D], BF16)
        nc.sync.dma_start(bias_sb[:], bias)
        nc.sync.dma_start(wpp_sb[:], wpp)
        nc.sync.dma_start(wp2z_sb[:], wp2z)

        # [hd, seq] layouts; pair heads stacked on partitions 0-63 / 64-127;
        # the h2 tensors hold the same head duplicated in both halves.
        QT_pair = qkpool.tile([128, N], BF16)
        KT_pair = qkpool.tile([128, N], BF16)
        QT_h2 = qkpool.tile([128, N], BF16)
        KT_h2 = qkpool.tile([128, N], BF16)

        # V natural [seq, hd] per head, chunked [128, 65] with a ones column.
        # Two half tiles per head (key chunks 0-15 / 16-31) so the attention
        # phase's whole-tile dependency doesn't wait for the last QKV chunk.
        V_aug = [
            [vpool.tile([128, (NCHUNK // 2) * 65], BF16, tag=f"vaug{h}_{s}",
                        name=f"vaug{h}_{s}") for s in range(2)]
            for h in range(3)
        ]

        def va_chunk(h, c):
            """AP of V_aug chunk c for head h: [128, 65]."""
            half, cc = divmod(c, NCHUNK // 2)
            return V_aug[h][half][:, 65 * cc: 65 * cc + 65]

        for h in range(3):
            for s in range(2):
                va3 = V_aug[h][s][:].rearrange("p (c m) -> p c m", m=65)
                nc.vector.memset(va3[:, :, 64], 1.0)

        # O^T (normalized) [feat, seq]: pair heads stacked; h2 zero-padded to
        # 128 partitions (rows 64-127 = 0) so proj matmuls get a 128-row
        # stationary (FWL).
        O_pair = opool_sb.tile([128, N], BF16)
        O_h2z = opool_sb.tile([128, N], BF16)
        nc.vector.memset(O_h2z[64:128, :], 0.0)

        # preload the exp activation table while the PE does QKV
        warm = const.tile([1, 1], F32)
        nc.vector.memset(warm[:], 0.0)
        nc.scalar.activation(warm[:], warm[:], AF.Exp)

        # ------------- stage A: QKV projection from pre-transposed x -------------
        with ExitStack() as bctx:
            wpool = bctx.enter_context(tc.tile_pool(name="wqkv", bufs=1))
            xtpool = bctx.enter_context(tc.tile_pool(name="xt", bufs=3))
            vtpool = bctx.enter_context(tc.tile_pool(name="vt", bufs=3))
            vstpool = bctx.enter_context(tc.tile_pool(name="vst", bufs=4))
            qkvps = bctx.enter_context(tc.tile_pool(name="qkv", bufs=4, space="PSUM"))

            wsb = wpool.tile([128, 5, KCH, 128], BF16)
            for g in (2, 4, 0, 1, 3):  # one DMA per group, in use order
                nc.sync.dma_start(wsb[:, g], wg[g].rearrange("(c p) m -> p c m", p=128))

            def qkv_chunk(j):
                jsl = bass.ts(j, 512)
                xt_j = xtpool.tile([128, KCH, 512], BF16, tag="xt", name=f"xt_{j}")
                nc.scalar.dma_start(xt_j[:], xt[j])
                vt_p = vtpool.tile([128, 512], BF16, tag="vtp", name=f"vtp_{j}")
                vt_2 = vtpool.tile([64, 512], BF16, tag="vt2", name=f"vt2_{j}")
                def qkv_group(g):
                    ps = qkvps.tile([128, 512], F32, tag="ps", name=f"ps_{j}_{g}")
                    for k in range(KCH):
                        nc.tensor.matmul(
                            ps[:], wsb[:, g, k, :], xt_j[:, k, :],
                            start=(k == 0), stop=(k == KCH - 1),
                        )
                    # bias-add + bf16 cast on the scalar engine
                    bcol = bias_sb[:, g: g + 1]
                    if g == 0:
                        nc.scalar.activation(QT_pair[:, jsl], ps[:], AF.Identity, bias=bcol)
                    elif g == 1:
                        nc.scalar.activation(KT_pair[:, jsl], ps[:], AF.Identity, bias=bcol)
                    elif g == 2:
                        nc.scalar.activation(vt_p[:], ps[:], AF.Identity, bias=bcol)
                    elif g == 3:
                        # [Q_h2 | K_h2] packed in one group
                        nc.scalar.activation(QT_h2[0:64, jsl], ps[0:64, :], AF.Identity,
                                             bias=bias_sb[0:64, 3:4])
                        nc.scalar.activation(KT_h2[64:128, jsl], ps[64:128, :], AF.Identity,
                                             bias=bias_sb[64:128, 3:4])
                    else:  # g == 4: V_h2, rows 0-63 only
                        nc.scalar.activation(vt_2[:], ps[0:64, :], AF.Identity,
                                             bias=bias_sb[0:64, 4:5])

                # V groups first so the V^T->V transpose chain (ACT copy ->
                # xbar DMA -> DVE copy) finishes before the attention phase
                # needs the (whole-tile-tracked) V_aug tiles.
                qkv_group(2)
                qkv_group(4)
                # V^T -> V natural via xbar transpose (contiguous staging; the
                # xbar mis-writes strided out APs on HW) then strided DVE copy
                half, c0 = divmod(4 * j, NCHUNK // 2)
                for h, src_ap in ((0, vt_p[0:64, :]), (1, vt_p[64:128, :]), (2, vt_2[:])):
                    vst = vstpool.tile([128, 4, 64], BF16, tag="vst", name=f"vst_{j}_{h}")
                    nc.sync.dma_start_transpose(vst[:], src_ap)
                    va3 = V_aug[h][half][:].rearrange("p (c m) -> p c m", m=65)
                    nc.gpsimd.tensor_copy(va3[:, c0:c0 + 4, 0:64], vst[:])
                qkv_group(0)
                qkv_group(1)
                qkv_group(3)

                # duplicate h2 Q/K into the other partition half for the
                # even/odd paired score matmuls
                nc.gpsimd.dma_start(QT_h2[64:128, jsl], QT_h2[0:64, jsl])
                nc.gpsimd.dma_start(KT_h2[0:64, jsl], KT_h2[64:128, jsl])

            for j in range(NSC):
                qkv_chunk(j)

        # ---------------- stage C: attention ----------------
        with ExitStack() as cctx:
            spool = cctx.enter_context(tc.tile_pool(name="s", bufs=3, space="PSUM"))
            opool = cctx.enter_context(tc.tile_pool(name="o", bufs=1, space="PSUM"))
            papool = cctx.enter_context(tc.tile_pool(name="pa", bufs=2))
            pipool = cctx.enter_context(tc.tile_pool(name="pi", bufs=2))
            osb_pool = cctx.enter_context(tc.tile_pool(name="osb", bufs=4))
            bcsb = cctx.enter_context(tc.tile_pool(name="bcs", bufs=2))
            dpool = cctx.enter_context(tc.tile_pool(name="dd", bufs=2))
            rpool = cctx.enter_context(tc.tile_pool(name="rr", bufs=2))
            rdpool = cctx.enter_context(tc.tile_pool(name="rd", bufs=2, space="DRAM"))
            ysb_pool = cctx.enter_context(tc.tile_pool(name="ysb", bufs=3))

            def softmax_p(s2, use_act, name):
                """exp(SCALE * s2) -> bf16 [128, 1024]; ACT or DVE variant."""
                if use_act:
                    p2 = papool.tile([128, 1024], BF16, tag="pa", name=f"pa_{name}")
                    nc.scalar.activation(p2[:], s2[:], AF.Exp, scale=SCALE)
                    return p2[:]
                pi = pipool.tile([128, 1024], I16, tag="pi", name=f"pi_{name}")
                nc.vector.tensor_scalar(pi[:], s2[:], EXP_C1, EXP_C2, ALU.mult, ALU.add)
                return pi[:].bitcast(BF16)

            def proj_subtile(pj, t4):
                # output projection of one 128-row q-subtile; borrows an s slot
                t = 4 * pj + t4
                tsl = bass.ts(t, 128)
                ysb = ysb_pool.tile([128, D], F32, tag="ysb", name=f"ysb_{t}")
                for half in range(2):
                    hsl = bass.ts(half, 384)
                    yp = spool.tile([128, 384], F32, tag="s2", name=f"yp_{t}_{half}")
                    nc.tensor.matmul(yp[:], O_pair[:, tsl], wpp_sb[:, hsl],
                                     start=True, stop=False)
                    nc.tensor.matmul(yp[:], O_h2z[:, tsl], wp2z_sb[:, hsl],
                                     start=False, stop=True)
                    nc.scalar.copy(ysb[:, hsl], yp[:])
                nc.sync.dma_start(y[128 * t: 128 * (t + 1), :], ysb[:])

            def normalize(jq, osb_group, h0):
                """Batched denominator reciprocal + DMA broadcast + scale for
                heads h0..h0+len(osb_group)-1 of query tile jq."""
                qsl = bass.ts(jq, 512)
                nh = len(osb_group)
                dd = dpool.tile([nh, 512], F32, tag="d3", name=f"d3_{jq}_{h0}")
                for i, o_sb in enumerate(osb_group):
                    nc.sync.dma_start(dd[i:i + 1, :], o_sb[64:65, :])
                rr = rpool.tile([nh, 512], F32, tag="r3", name=f"r3_{jq}_{h0}")
                nc.vector.reciprocal_approx_fast(rr[:], dd[:])
                rdd = rdpool.tile([nh, 512], F32, tag="rd3", name=f"rd3_{jq}_{h0}")
                nc.gpsimd.dma_start(rdd[:], rr[:])
                for i, o_sb in enumerate(osb_group):
                    h = h0 + i
                    bcs = bcsb.tile([64, 512], F32, tag="bcs", name=f"bcs_{jq}_{h}")
                    nc.gpsimd.dma_start(bcs[:], rdd[i:i + 1, :].to_broadcast([64, 512]))
                    if h < 2:
                        dest = O_pair[64 * h: 64 * (h + 1), qsl]
                    else:
                        dest = O_h2z[0:64, qsl]
                    nc.vector.tensor_mul(dest, o_sb[0:64, :], bcs[:])

            pending = []
            for jq in range(NQT):
                qsl = bass.ts(jq, 512)
                osbs = []

                def mk_s2_pair(c):
                    ksl = bass.ts(c, 128)
                    s2 = spool.tile([128, 1024], F32, tag="s2", name=f"s2_{jq}_{c}")
                    nc.tensor.matmul(s2[:, 0:512], KT_pair[0:64, ksl], QT_pair[0:64, qsl],
                                     start=True, stop=True)
                    nc.tensor.matmul(s2[:, 512:1024], KT_pair[64:128, ksl], QT_pair[64:128, qsl],
                                     start=True, stop=True)
                    return s2

                def mk_s2_h2(cc):
                    s2 = spool.tile([128, 1024], F32, tag="s2", name=f"s2h_{jq}_{cc}")
                    nc.tensor.matmul(s2[:, 0:512], KT_h2[0:64, bass.ts(2 * cc, 128)], QT_h2[0:64, qsl],
                                     start=True, stop=True)
                    nc.tensor.matmul(s2[:, 512:1024], KT_h2[64:128, bass.ts(2 * cc + 1, 128)], QT_h2[64:128, qsl],
                                     start=True, stop=True)
                    return s2

                # heads h0/h1: same key chunk in the two PE row groups.
                # Software-pipelined emit order: the next chunk's (independent)
                # score matmuls are queued BEFORE this chunk's PV matmuls so
                # the in-order PE never head-blocks on the exp result.
                o0 = opool.tile([65, 512], F32, tag="o0")
                o1 = opool.tile([65, 512], F32, tag="o1")
                s2 = mk_s2_pair(0)
                for c in range(NCHUNK):
                    if pending and pending[0] <= jq - 2:
                        if c in (6, 13, 20, 27):
                            pj = pending[0]
                            proj_subtile(pj, (c - 6) // 7)
                            if c == 27:
                                pending.pop(0)
                    pv = softmax_p(s2, use_act=(c % 2 == 0), name=f"{jq}_{c}")
                    if c + 1 < NCHUNK:
                        s2 = mk_s2_pair(c + 1)
                    nc.tensor.matmul(o0[:], va_chunk(0, c), pv[:, 0:512],
                                     start=(c == 0), stop=(c == NCHUNK - 1))
                    nc.tensor.matmul(o1[:], va_chunk(1, c), pv[:, 512:1024],
                                     start=(c == 0), stop=(c == NCHUNK - 1))

                # head h2: even/odd key chunks in the two row groups
                s2 = mk_s2_h2(0)
                # free the o0/o1 PSUM banks quickly; normalization is deferred
                for h, o_ps in ((0, o0), (1, o1)):
                    o_sb = osb_pool.tile([65, 512], F32, tag="osb", name=f"osb_{jq}_{h}")
                    nc.scalar.copy(o_sb[:], o_ps[:])
                    osbs.append(o_sb)
                # normalize h0/h1 now -- their recip/broadcast DMA chain hides
                # under the h2 loop instead of adding to the per-jq tail
                normalize(jq, osbs[0:2], 0)
                o2 = opool.tile([65, 512], F32, tag="o0")
                for cc in range(NCHUNK // 2):
                    ce, co = 2 * cc, 2 * cc + 1
                    if pending and pending[0] <= jq - 1:
                        if cc in (3, 7, 11, 15):
                            pj = pending[0]
                            proj_subtile(pj, {3: 0, 7: 1, 11: 2, 15: 3}[cc])
                            if cc == 15:
                                pending.pop(0)
                    pv = softmax_p(s2, use_act=(cc % 2 == 0), name=f"h2_{jq}_{cc}")
                    if cc + 1 < NCHUNK // 2:
                        s2 = mk_s2_h2(cc + 1)
                    nc.tensor.matmul(o2[:], va_chunk(2, ce), pv[:, 0:512],
                                     start=(cc == 0), stop=False)
                    nc.tensor.matmul(o2[:], va_chunk(2, co), pv[:, 512:1024],
                                     start=False, stop=(cc == NCHUNK // 2 - 1))
                o_sb = osb_pool.tile([65, 512], F32, tag="osb", name=f"osb_{jq}_2")
                nc.scalar.copy(o_sb[:], o2[:])
                osbs.append(o_sb)
                normalize(jq, osbs[2:3], 2)
                pending.append(jq)

            for pj in pending:
                for t4 in range(4):
                    proj_subtile(pj, t4)

    nc.compile()
    return nc


_PROGRAM = None


def _get_program():
    global _PROGRAM
    if _PROGRAM is None:
        _PROGRAM = build_program()
    return _PROGRAM


def make_core_inputs(x, W_qkv, b_qkv, W_proj):
    """Per-core input dicts implementing the (batch, head-group) sharding."""
    x = np.ascontiguousarray(np.asarray(x, np.float32))
    W_qkv = np.asarray(W_qkv, np.float32)
    b_qkv = np.asarray(b_qkv, np.float32)
    W_proj = np.asarray(W_proj, np.float32)
    bf = ml_dtypes.bfloat16

    # xt[b][j, p, k, m] = x[b, 512j + m, 128k + p]
    xts = []
    for b in range(B):
        xb = x[b].astype(bf)                               # [N, D]
        xt = xb.reshape(NSC, 512, KCH, 128)                # [j, m, k, p]
        xt = np.ascontiguousarray(xt.transpose(0, 3, 2, 1))  # [j, p, k, m]
        xts.append(xt)

    ins = []
    for c in range(NC):
        b, g = divmod(c, 4)
        col = 192 * g
        wgq = np.zeros((5, D, 128), np.float32)
        bias = np.zeros((128, 5), np.float32)
        for i, off in enumerate((0, D, 2 * D)):  # q, k, v pair-head blocks
            wgq[i] = W_qkv[:, off + col: off + col + 128]
            bias[:, i] = b_qkv[off + col: off + col + 128]
        q2 = W_qkv[:, col + 128: col + 192]
        k2 = W_qkv[:, D + col + 128: D + col + 192]
        v2 = W_qkv[:, 2 * D + col + 128: 2 * D + col + 192]
        wgq[3] = np.concatenate([q2, k2], axis=1)
        wgq[4, :, 0:64] = v2
        bias[0:64, 3] = b_qkv[col + 128: col + 192]
        bias[64:128, 3] = b_qkv[D + col + 128: D + col + 192]
        bias[0:64, 4] = b_qkv[2 * D + col + 128: 2 * D + col + 192]

        wp2z = np.zeros((128, D), np.float32)
        wp2z[0:64, :] = W_proj[col + 128: col + 192, :]

        ins.append({
            "xt": xts[b],
            "wg": wgq.astype(bf),
            "bias": bias,
            "wpp": W_proj[col: col + 128, :].astype(bf),
            "wp2z": wp2z.astype(bf),
        })
    return ins


def gather_output(results, b_proj):
    b_proj = np.asarray(b_proj, np.float32)
    outs = []
    for b in range(B):
        acc = results[4 * b]["y"].astype(np.float32).copy()
        for c in range(4 * b + 1, 4 * b + 4):
            acc += results[c]["y"]
        outs.append(acc + b_proj)
    return np.stack(outs).astype(np.float32)


def kernel(x, W_qkv, b_qkv, W_proj, b_proj):
    ins = make_core_inputs(x, W_qkv, b_qkv, W_proj)
    prog = _get_program()
    res = run_bass_kernel_spmd(prog, ins, core_ids=list(range(NC)))
    return gather_output(res.results, b_proj)


# revision 21
# speedup vs baseline: 1.0296x; 1.0296x over previous
"""Multi-head attention (B=2, N=4096, D=768, H=12) on 8 Trainium2 NeuronCores.

Sharding: core c handles batch b = c//4 and heads [3g, 3g+1, 3g+2] with
g = c%4 (data parallel on B, head parallel on H). Each core computes its
heads' Q/K/V from x[b], runs softmax attention, and produces the partial
output projection for its head block; the host sums the 4 partials per
batch (row-parallel unshard) and adds b_proj.

v2 changes vs the 674us baseline (which was exp/ACT-throughput bound at
~1205ns per key-chunk with the PE at ~905ns):
  - x^T is pre-transposed and pre-cast to bf16 on the host, removing the
    in-kernel fp32 x load + DVE cast + xbar transpose (~40us of startup
    latency and DMA/DVE pressure).
  - softmax exp alternates between the ACT engine (native Exp) and the
    DVE (Schraudolph bit-trick: p = bitcast_bf16(int16(s*c1 + c2)),
    one tensor_scalar op), halving the softmax-stage cost so the
    attention phase runs at the PE's ~905ns/chunk cadence.
  - softmax denominator reciprocal via reciprocal_approx_fast on a
    [3, 512] batched tile (the baseline's [1,512] nc.vector.reciprocal
    ran single-lane at 8 cyc/elem: 95us -> ~7us).
  - QKV projection drops the duplicated-h2 matmul group (5 groups
    instead of 6); the h2 Q/K partition-duplicates for the even/odd
    score pairing are made with two small SBUF->SBUF DMAs per chunk.
  - O_h2 is stored zero-padded to 128 partitions so the projection
    matmul runs with a 128-row stationary (FWL-eligible).
  - proj PSUM->SBUF copies moved from DVE to the scalar engine.
"""

import numpy as np
import ml_dtypes
from contextlib import ExitStack

import concourse.bass as bass
from concourse import bacc
import concourse.tile as tile
import concourse.mybir as mybir
from concourse.bass_utils import run_bass_kernel_spmd

F32 = mybir.dt.float32
BF16 = mybir.dt.bfloat16
I16 = mybir.dt.int16
AF = mybir.ActivationFunctionType
ALU = mybir.AluOpType

B, N, D, H, HD = 2, 4096, 768, 12, 64
SCALE = HD ** -0.5
NC = 8
NCHUNK = N // 128          # 32 key chunks of 128
NQT = N // 512             # 8 query tiles of 512
NSC = N // 512             # 8 seq chunks of 512 (QKV stage)
KCH = D // 128             # 6 contraction chunks

# Schraudolph exp for the DVE half of the softmax:
#   p = bitcast_bf16(int16(s * EXP_C1 + EXP_C2)) ~= exp(s * SCALE)
# sigma = -5.25 balances the max relative error (~3.3%) for either
# truncating or round-to-nearest float->int conversion.
EXP_C1 = float(SCALE * np.log2(np.e) * 128.0)
EXP_C2 = 16256.0 - 5.25


def build_program():
    nc = bacc.Bacc("TRN2", target_bir_lowering=False, debug=False)

    # xt[j, p, k, m] = x[512j + m, 128k + p] in bf16 (host pre-transposed)
    xt = nc.dram_tensor("xt", [NSC, 128, KCH, 512], BF16, kind="ExternalInput").ap()
    wg = nc.dram_tensor("wg", [5, D, 128], BF16, kind="ExternalInput").ap()
    bias = nc.dram_tensor("bias", [128, 5], F32, kind="ExternalInput").ap()
    wpp = nc.dram_tensor("wpp", [128, D], BF16, kind="ExternalInput").ap()
    wp2z = nc.dram_tensor("wp2z", [128, D], BF16, kind="ExternalInput").ap()
    y = nc.dram_tensor("y", [N, D], F32, kind="ExternalOutput").ap()

    with tile.TileContext(nc) as tc, ExitStack() as octx:
        const = octx.enter_context(tc.tile_pool(name="const", bufs=1))
        qkpool = octx.enter_context(tc.tile_pool(name="qk", bufs=1))
        vpool = octx.enter_context(tc.tile_pool(name="vaug", bufs=1))
        opool_sb = octx.enter_context(tc.tile_pool(name="onorm", bufs=1))

        bias_sb = const.tile([128, 5], F32)
        wpp_sb = const.tile([128, D], BF16)
        wp2z_sb = const.tile([128, D], BF16)
        nc.sync.dma_start(bias_sb[:], bias)
        nc.sync.dma_start(wpp_sb[:], wpp)
        nc.sync.dma_start(wp2z_sb[:], wp2z)

        # [hd, seq] layouts; pair heads stacked on partitions 0-63 / 64-127;
        # the h2 tensors hold the same head duplicated in both halves.
        QT_pair = qkpool.tile([128, N], BF16)
        KT_pair = qkpool.tile([128, N], BF16)
        QT_h2 = qkpool.tile([128, N], BF16)
        KT_h2 = qkpool.tile([128, N], BF16)

        # V natural [seq, hd] per head, chunked [128, 65] with a ones column.
        # Two half tiles per head (key chunks 0-15 / 16-31) so the attention
        # phase's whole-tile dependency doesn't wait for the last QKV chunk.
        V_aug = [
            [vpool.tile([128, (NCHUNK // 2) * 65], BF16, tag=f"vaug{h}_{s}",
                        name=f"vaug{h}_{s}") for s in range(2)]
            for h in range(3)
        ]

        def va_chunk(h, c):
            """AP of V_aug chunk c for head h: [128, 65]."""
            half, cc = divmod(c, NCHUNK // 2)
            return V_aug[h][half][:, 65 * cc: 65 * cc + 65]

        for h in range(3):
            for s in range(2):
                va3 = V_aug[h][s][:].rearrange("p (c m) -> p c m", m=65)
                nc.vector.memset(va3[:, :, 64], 1.0)

        # O^T (normalized) [feat, seq]: pair heads stacked; h2 zero-padded to
        # 128 partitions (rows 64-127 = 0) so proj matmuls get a 128-row
        # stationary (FWL).
        O_pair = opool_sb.tile([128, N], BF16)
        O_h2z = opool_sb.tile([128, N], BF16)
        nc.vector.memset(O_h2z[64:128, :], 0.0)

        # preload the exp activation table while the PE does QKV
        warm = const.tile([1, 1], F32)
        nc.vector.memset(warm[:], 0.0)
        nc.scalar.activation(warm[:], warm[:], AF.Exp)

        # ------------- stage A: QKV projection from pre-transposed x -------------
        with ExitStack() as bctx:
            wpool = bctx.enter_context(tc.tile_pool(name="wqkv", bufs=1))
            xtpool = bctx.enter_context(tc.tile_pool(name="xt", bufs=3))
            vtpool = bctx.enter_context(tc.tile_pool(name="vt", bufs=3))
            vstpool = bctx.enter_context(tc.tile_pool(name="vst", bufs=4))
            qkvps = bctx.enter_context(tc.tile_pool(name="qkv", bufs=4, space="PSUM"))

            wsb = wpool.tile([128, 5, KCH, 128], BF16)
            for g in (2, 4, 0, 1, 3):  # one DMA per group, in use order
                nc.sync.dma_start(wsb[:, g], wg[g].rearrange("(c p) m -> p c m", p=128))

            def qkv_chunk(j):
                jsl = bass.ts(j, 512)
                xt_j = xtpool.tile([128, KCH, 512], BF16, tag="xt", name=f"xt_{j}")
                nc.sync.dma_start(xt_j[:], xt[j])
                vt_p = vtpool.tile([128, 512], BF16, tag="vtp", name=f"vtp_{j}")
                vt_2 = vtpool.tile([64, 512], BF16, tag="vt2", name=f"vt2_{j}")
                def qkv_group(g):
                    ps = qkvps.tile([128, 512], F32, tag="ps", name=f"ps_{j}_{g}")
                    for k in range(KCH):
                        nc.tensor.matmul(
                            ps[:], wsb[:, g, k, :], xt_j[:, k, :],
                            start=(k == 0), stop=(k == KCH - 1),
                        )
                    # bias-add + bf16 cast on the scalar engine
                    bcol = bias_sb[:, g: g + 1]
                    if g == 0:
                        nc.scalar.activation(QT_pair[:, jsl], ps[:], AF.Identity, bias=bcol)
                    elif g == 1:
                        nc.scalar.activation(KT_pair[:, jsl], ps[:], AF.Identity, bias=bcol)
                    elif g == 2:
                        nc.scalar.activation(vt_p[:], ps[:], AF.Identity, bias=bcol)
                    elif g == 3:
                        # [Q_h2 | K_h2] packed in one group
                        nc.scalar.activation(QT_h2[0:64, jsl], ps[0:64, :], AF.Identity,
                                             bias=bias_sb[0:64, 3:4])
                        nc.scalar.activation(KT_h2[64:128, jsl], ps[64:128, :], AF.Identity,
                                             bias=bias_sb[64:128, 3:4])
                    else:  # g == 4: V_h2, rows 0-63 only
                        nc.scalar.activation(vt_2[:], ps[0:64, :], AF.Identity,
                                             bias=bias_sb[0:64, 4:5])

                # V groups first so the V^T->V transpose chain (ACT copy ->
                # xbar DMA -> DVE copy) finishes before the attention phase
                # needs the (whole-tile-tracked) V_aug tiles.
                qkv_group(2)
                qkv_group(4)
                # V^T -> V natural via xbar transpose (contiguous staging; the
                # xbar mis-writes strided out APs on HW) then strided DVE copy
                half, c0 = divmod(4 * j, NCHUNK // 2)
                for h, src_ap in ((0, vt_p[0:64, :]), (1, vt_p[64:128, :]), (2, vt_2[:])):
                    vst = vstpool.tile([128, 4, 64], BF16, tag="vst", name=f"vst_{j}_{h}")
                    nc.sync.dma_start_transpose(vst[:], src_ap)
                    va3 = V_aug[h][half][:].rearrange("p (c m) -> p c m", m=65)
                    nc.vector.tensor_copy(va3[:, c0:c0 + 4, 0:64], vst[:])
                qkv_group(0)
                qkv_group(1)
                qkv_group(3)

                # duplicate h2 Q/K into the other partition half for the
                # even/odd paired score matmuls
                nc.gpsimd.dma_start(QT_h2[64:128, jsl], QT_h2[0:64, jsl])
                nc.gpsimd.dma_start(KT_h2[0:64, jsl], KT_h2[64:128, jsl])

            for j in range(NSC):
                qkv_chunk(j)

        # ---------------- stage C: attention ----------------
        with ExitStack() as cctx:
            spool = cctx.enter_context(tc.tile_pool(name="s", bufs=3, space="PSUM"))
            opool = cctx.enter_context(tc.tile_pool(name="o", bufs=1, space="PSUM"))
            papool = cctx.enter_context(tc.tile_pool(name="pa", bufs=2))
            pipool = cctx.enter_context(tc.tile_pool(name="pi", bufs=2))
            osb_pool = cctx.enter_context(tc.tile_pool(name="osb", bufs=4))
            bcsb = cctx.enter_context(tc.tile_pool(name="bcs", bufs=2))
            dpool = cctx.enter_context(tc.tile_pool(name="dd", bufs=2))
            rpool = cctx.enter_context(tc.tile_pool(name="rr", bufs=2))
            rdpool = cctx.enter_context(tc.tile_pool(name="rd", bufs=2, space="DRAM"))
            ysb_pool = cctx.enter_context(tc.tile_pool(name="ysb", bufs=3))

            def softmax_p(s2, use_act, name):
                """exp(SCALE * s2) -> bf16 [128, 1024]; ACT or DVE variant."""
                if use_act:
                    p2 = papool.tile([128, 1024], BF16, tag="pa", name=f"pa_{name}")
                    nc.scalar.activation(p2[:], s2[:], AF.Exp, scale=SCALE)
                    return p2[:]
                pi = pipool.tile([128, 1024], I16, tag="pi", name=f"pi_{name}")
                nc.vector.tensor_scalar(pi[:], s2[:], EXP_C1, EXP_C2, ALU.mult, ALU.add)
                return pi[:].bitcast(BF16)

            def proj_subtile(pj, t4):
                # output projection of one 128-row q-subtile; borrows an s slot
                t = 4 * pj + t4
                tsl = bass.ts(t, 128)
                ysb = ysb_pool.tile([128, D], F32, tag="ysb", name=f"ysb_{t}")
                for half in range(2):
                    hsl = bass.ts(half, 384)
                    yp = spool.tile([128, 384], F32, tag="s2", name=f"yp_{t}_{half}")
                    nc.tensor.matmul(yp[:], O_pair[:, tsl], wpp_sb[:, hsl],
                                     start=True, stop=False)
                    nc.tensor.matmul(yp[:], O_h2z[:, tsl], wp2z_sb[:, hsl],
                                     start=False, stop=True)
                    nc.scalar.copy(ysb[:, hsl], yp[:])
                nc.sync.dma_start(y[128 * t: 128 * (t + 1), :], ysb[:])

            def normalize(jq, osb_group, h0):
                """Batched denominator reciprocal + DMA broadcast + scale for
                heads h0..h0+len(osb_group)-1 of query tile jq."""
                qsl = bass.ts(jq, 512)
                nh = len(osb_group)
                dd = dpool.tile([nh, 512], F32, tag="d3", name=f"d3_{jq}_{h0}")
                for i, o_sb in enumerate(osb_group):
                    nc.sync.dma_start(dd[i:i + 1, :], o_sb[64:65, :])
                rr = rpool.tile([nh, 512], F32, tag="r3", name=f"r3_{jq}_{h0}")
                nc.vector.reciprocal_approx_fast(rr[:], dd[:])
                rdd = rdpool.tile([nh, 512], F32, tag="rd3", name=f"rd3_{jq}_{h0}")
                nc.gpsimd.dma_start(rdd[:], rr[:])
                for i, o_sb in enumerate(osb_group):
                    h = h0 + i
                    bcs = bcsb.tile([64, 512], F32, tag="bcs", name=f"bcs_{jq}_{h}")
                    nc.gpsimd.dma_start(bcs[:], rdd[i:i + 1, :].to_broadcast([64, 512]))
                    if h < 2:
                        dest = O_pair[64 * h: 64 * (h + 1), qsl]
                    else:
                        dest = O_h2z[0:64, qsl]
                    nc.vector.tensor_mul(dest, o_sb[0:64, :], bcs[:])

            pending = []
            for jq in range(NQT):
                qsl = bass.ts(jq, 512)
                osbs = []

                def mk_s2_pair(c):
                    ksl = bass.ts(c, 128)
                    s2 = spool.tile([128, 1024], F32, tag="s2", name=f"s2_{jq}_{c}")
                    nc.tensor.matmul(s2[:, 0:512], KT_pair[0:64, ksl], QT_pair[0:64, qsl],
                                     start=True, stop=True)
                    nc.tensor.matmul(s2[:, 512:1024], KT_pair[64:128, ksl], QT_pair[64:128, qsl],
                                     start=True, stop=True)
                    return s2

                def mk_s2_h2(cc):
                    s2 = spool.tile([128, 1024], F32, tag="s2", name=f"s2h_{jq}_{cc}")
                    nc.tensor.matmul(s2[:, 0:512], KT_h2[0:64, bass.ts(2 * cc, 128)], QT_h2[0:64, qsl],
                                     start=True, stop=True)
                    nc.tensor.matmul(s2[:, 512:1024], KT_h2[64:128, bass.ts(2 * cc + 1, 128)], QT_h2[64:128, qsl],
                                     start=True, stop=True)
                    return s2

                # heads h0/h1: same key chunk in the two PE row groups.
                # Software-pipelined emit order: the next chunk's (independent)
                # score matmuls are queued BEFORE this chunk's PV matmuls so
                # the in-order PE never head-blocks on the exp result.
                o0 = opool.tile([65, 512], F32, tag="o0")
                o1 = opool.tile([65, 512], F32, tag="o1")
                s2 = mk_s2_pair(0)
                for c in range(NCHUNK):
                    if pending and pending[0] <= jq - 2:
                        if c in (6, 13, 20, 27):
                            pj = pending[0]
                            proj_subtile(pj, (c - 6) // 7)
                            if c == 27:
                                pending.pop(0)
                    pv = softmax_p(s2, use_act=(c % 2 == 0) or (jq == 0 and c < 8),
                                   name=f"{jq}_{c}")
                    if c + 1 < NCHUNK:
                        s2 = mk_s2_pair(c + 1)
                    nc.tensor.matmul(o0[:], va_chunk(0, c), pv[:, 0:512],
                                     start=(c == 0), stop=(c == NCHUNK - 1))
                    nc.tensor.matmul(o1[:], va_chunk(1, c), pv[:, 512:1024],
                                     start=(c == 0), stop=(c == NCHUNK - 1))

                # head h2: even/odd key chunks in the two row groups
                s2 = mk_s2_h2(0)
                # free the o0/o1 PSUM banks quickly; normalization is deferred
                for h, o_ps in ((0, o0), (1, o1)):
                    o_sb = osb_pool.tile([65, 512], F32, tag="osb", name=f"osb_{jq}_{h}")
                    nc.scalar.copy(o_sb[:], o_ps[:])
                    osbs.append(o_sb)
                # normalize h0/h1 now -- their recip/broadcast DMA chain hides
                # under the h2 loop instead of adding to the per-jq tail
                normalize(jq, osbs[0:2], 0)
                o2 = opool.tile([65, 512], F32, tag="o0")
                for cc in range(NCHUNK // 2):
                    ce, co = 2 * cc, 2 * cc + 1
                    if pending and pending[0] <= jq - 1:
                        if cc in (3, 7, 11, 15):
                            pj = pending[0]
                            proj_subtile(pj, {3: 0, 7: 1, 11: 2, 15: 3}[cc])
                            if cc == 15:
                                pending.pop(0)
                    pv = softmax_p(s2, use_act=(cc % 2 == 0), name=f"h2_{jq}_{cc}")
                    if cc + 1 < NCHUNK // 2:
                        s2 = mk_s2_h2(cc + 1)
                    nc.tensor.matmul(o2[:], va_chunk(2, ce), pv[:, 0:512],
                                     start=(cc == 0), stop=False)
                    nc.tensor.matmul(o2[:], va_chunk(2, co), pv[:, 512:1024],
                                     start=False, stop=(cc == NCHUNK // 2 - 1))
                o_sb = osb_pool.tile([65, 512], F32, tag="osb", name=f"osb_{jq}_2")
                nc.scalar.copy(o_sb[:], o2[:])
                osbs.append(o_sb)
                normalize(jq, osbs[2:3], 2)
                pending.append(jq)

            for pj in pending:
                for t4 in range(4):
                    proj_subtile(pj, t4)

    nc.compile()
    return nc


_PROGRAM = None


def _get_program():
    global _PROGRAM
    if _PROGRAM is None:
        _PROGRAM = build_program()
    return _PROGRAM


def make_core_inputs(x, W_qkv, b_qkv, W_proj):
    """Per-core input dicts implementing the (batch, head-group) sharding."""
    x = np.ascontiguousarray(np.asarray(x, np.float32))
    W_qkv = np.asarray(W_qkv, np.float32)
    b_qkv = np.asarray(b_qkv, np.float32)
    W_proj = np.asarray(W_proj, np.float32)
    bf = ml_dtypes.bfloat16

    # xt[b][j, p, k, m] = x[b, 512j + m, 128k + p]
    xts = []
    for b in range(B):
        xb = x[b].astype(bf)                               # [N, D]
        xt = xb.reshape(NSC, 512, KCH, 128)                # [j, m, k, p]
        xt = np.ascontiguousarray(xt.transpose(0, 3, 2, 1))  # [j, p, k, m]
        xts.append(xt)

    ins = []
    for c in range(NC):
        b, g = divmod(c, 4)
        col = 192 * g
        wgq = np.zeros((5, D, 128), np.float32)
        bias = np.zeros((128, 5), np.float32)
        for i, off in enumerate((0, D, 2 * D)):  # q, k, v pair-head blocks
            wgq[i] = W_qkv[:, off + col: off + col + 128]
            bias[:, i] = b_qkv[off + col: off + col + 128]
        q2 = W_qkv[:, col + 128: col + 192]
        k2 = W_qkv[:, D + col + 128: D + col + 192]
        v2 = W_qkv[:, 2 * D + col + 128: 2 * D + col + 192]
        wgq[3] = np.concatenate([q2, k2], axis=1)
        wgq[4, :, 0:64] = v2
        bias[0:64, 3] = b_qkv[col + 128: col + 192]
        bias[64:128, 3] = b_qkv[D + col + 128: D + col + 192]
        bias[0:64, 4] = b_qkv[2 * D + col + 128: 2 * D + col + 192]

        wp2z = np.zeros((128, D), np.float32)
        wp2z[0:64, :] = W_proj[col + 128: col + 192, :]

        ins.append({
            "xt": xts[b],
            "wg": wgq.astype(bf),
            "bias": bias,
            "wpp": W_proj[col: col + 128, :].astype(bf),
            "wp2z": wp2z.astype(bf),
        })
    return ins


def gather_output(results, b_proj):
    b_proj = np.asarray(b_proj, np.float32)
    outs = []
    for b in range(B):
        acc = results[4 * b]["y"].astype(np.float32).copy()
        for c in range(4 * b + 1, 4 * b + 4):
            acc += results[c]["y"]
        outs.append(acc + b_proj)
    return np.stack(outs).astype(np.float32)


def kernel(x, W_qkv, b_qkv, W_proj, b_proj):
    ins = make_core_inputs(x, W_qkv, b_qkv, W_proj)
    prog = _get_program()
    res = run_bass_kernel_spmd(prog, ins, core_ids=list(range(NC)))
    return gather_output(res.results, b_proj)


# revision 22
# speedup vs baseline: 1.0514x; 1.0212x over previous
"""Multi-head attention (B=2, N=4096, D=768, H=12) on 8 Trainium2 NeuronCores.

Sharding: core c handles batch b = c//4 and heads [3g, 3g+1, 3g+2] with
g = c%4 (data parallel on B, head parallel on H). Each core computes its
heads' Q/K/V from x[b], runs softmax attention, and produces the partial
output projection for its head block; the host sums the 4 partials per
batch (row-parallel unshard) and adds b_proj.

v2 changes vs the 674us baseline (which was exp/ACT-throughput bound at
~1205ns per key-chunk with the PE at ~905ns):
  - x^T is pre-transposed and pre-cast to bf16 on the host, removing the
    in-kernel fp32 x load + DVE cast + xbar transpose (~40us of startup
    latency and DMA/DVE pressure).
  - softmax exp alternates between the ACT engine (native Exp) and the
    DVE (Schraudolph bit-trick: p = bitcast_bf16(int16(s*c1 + c2)),
    one tensor_scalar op), halving the softmax-stage cost so the
    attention phase runs at the PE's ~905ns/chunk cadence.
  - softmax denominator reciprocal via reciprocal_approx_fast on a
    [3, 512] batched tile (the baseline's [1,512] nc.vector.reciprocal
    ran single-lane at 8 cyc/elem: 95us -> ~7us).
  - QKV projection drops the duplicated-h2 matmul group (5 groups
    instead of 6); the h2 Q/K partition-duplicates for the even/odd
    score pairing are made with two small SBUF->SBUF DMAs per chunk.
  - O_h2 is stored zero-padded to 128 partitions so the projection
    matmul runs with a 128-row stationary (FWL-eligible).
  - proj PSUM->SBUF copies moved from DVE to the scalar engine.
"""

import numpy as np
import ml_dtypes
from contextlib import ExitStack

import concourse.bass as bass
from concourse import bacc
import concourse.tile as tile
import concourse.mybir as mybir
from concourse.bass_utils import run_bass_kernel_spmd

F32 = mybir.dt.float32
BF16 = mybir.dt.bfloat16
I16 = mybir.dt.int16
AF = mybir.ActivationFunctionType
ALU = mybir.AluOpType

B, N, D, H, HD = 2, 4096, 768, 12, 64
SCALE = HD ** -0.5
NC = 8
NCHUNK = N // 128          # 32 key chunks of 128
NQT = N // 512             # 8 query tiles of 512
NSC = N // 512             # 8 seq chunks of 512 (QKV stage)
KCH = D // 128             # 6 contraction chunks

# Schraudolph exp for the DVE half of the softmax:
#   p = bitcast_bf16(int16(s * EXP_C1 + EXP_C2)) ~= exp(s * SCALE)
# sigma = -5.25 balances the max relative error (~3.3%) for either
# truncating or round-to-nearest float->int conversion.
EXP_C1 = float(SCALE * np.log2(np.e) * 128.0)
EXP_C2 = 16256.0 - 5.25


def build_program():
    nc = bacc.Bacc("TRN2", target_bir_lowering=False, debug=False)

    # xt[j, p, k, m] = x[512j + m, 128k + p] in bf16 (host pre-transposed)
    xt = nc.dram_tensor("xt", [NSC, 128, KCH, 512], BF16, kind="ExternalInput").ap()
    wg = nc.dram_tensor("wg", [5, D, 128], BF16, kind="ExternalInput").ap()
    bias = nc.dram_tensor("bias", [128, 5], F32, kind="ExternalInput").ap()
    wpp = nc.dram_tensor("wpp", [128, D], BF16, kind="ExternalInput").ap()
    wp2z = nc.dram_tensor("wp2z", [128, D], BF16, kind="ExternalInput").ap()
    y = nc.dram_tensor("y", [N, D], F32, kind="ExternalOutput").ap()

    with tile.TileContext(nc) as tc, ExitStack() as octx:
        const = octx.enter_context(tc.tile_pool(name="const", bufs=1))
        qkpool = octx.enter_context(tc.tile_pool(name="qk", bufs=1))
        vpool = octx.enter_context(tc.tile_pool(name="vaug", bufs=1))
        opool_sb = octx.enter_context(tc.tile_pool(name="onorm", bufs=1))

        bias_sb = const.tile([128, 5], F32)
        wpp_sb = const.tile([128, D], BF16)
        wp2z_sb = const.tile([128, D], BF16)
        nc.sync.dma_start(bias_sb[:], bias)
        nc.sync.dma_start(wpp_sb[:], wpp)
        nc.sync.dma_start(wp2z_sb[:], wp2z)

        # [hd, seq] layouts; pair heads stacked on partitions 0-63 / 64-127;
        # the h2 tensors hold the same head duplicated in both halves.
        QT_pair = qkpool.tile([128, N], BF16)
        KT_pair = qkpool.tile([128, N], BF16)
        QT_h2 = qkpool.tile([128, N], BF16)
        KT_h2 = qkpool.tile([128, N], BF16)

        # V natural [seq, hd] per head, chunked [128, 65] with a ones column.
        # Two half tiles per head (key chunks 0-15 / 16-31) so the attention
        # phase's whole-tile dependency doesn't wait for the last QKV chunk.
        V_aug = [
            [vpool.tile([128, (NCHUNK // 2) * 65], BF16, tag=f"vaug{h}_{s}",
                        name=f"vaug{h}_{s}") for s in range(2)]
            for h in range(3)
        ]

        def va_chunk(h, c):
            """AP of V_aug chunk c for head h: [128, 65]."""
            half, cc = divmod(c, NCHUNK // 2)
            return V_aug[h][half][:, 65 * cc: 65 * cc + 65]

        for h in range(3):
            for s in range(2):
                va3 = V_aug[h][s][:].rearrange("p (c m) -> p c m", m=65)
                nc.vector.memset(va3[:, :, 64], 1.0)

        # O^T (normalized) [feat, seq]: pair heads stacked; h2 zero-padded to
        # 128 partitions (rows 64-127 = 0) so proj matmuls get a 128-row
        # stationary (FWL).
        O_pair = opool_sb.tile([128, N], BF16)
        O_h2z = opool_sb.tile([128, N], BF16)
        nc.vector.memset(O_h2z[64:128, :], 0.0)

        # preload the exp activation table while the PE does QKV
        warm = const.tile([1, 1], F32)
        nc.vector.memset(warm[:], 0.0)
        nc.scalar.activation(warm[:], warm[:], AF.Exp)

        # ------------- stage A: QKV projection from pre-transposed x -------------
        with ExitStack() as bctx:
            wpool = bctx.enter_context(tc.tile_pool(name="wqkv", bufs=1))
            xtpool = bctx.enter_context(tc.tile_pool(name="xt", bufs=3))
            vtpool = bctx.enter_context(tc.tile_pool(name="vt", bufs=3))
            vstpool = bctx.enter_context(tc.tile_pool(name="vst", bufs=4))
            qkvps = bctx.enter_context(tc.tile_pool(name="qkv", bufs=4, space="PSUM"))

            wsb = wpool.tile([128, 5, KCH, 128], BF16)
            for g in (2, 4, 0, 1, 3):  # one DMA per group, in use order
                nc.sync.dma_start(wsb[:, g], wg[g].rearrange("(c p) m -> p c m", p=128))

            def qkv_chunk(j):
                jsl = bass.ts(j, 512)
                xt_j = xtpool.tile([128, KCH, 512], BF16, tag="xt", name=f"xt_{j}")
                nc.sync.dma_start(xt_j[:], xt[j])
                vt_p = vtpool.tile([128, 512], BF16, tag="vtp", name=f"vtp_{j}")
                vt_2 = vtpool.tile([64, 512], BF16, tag="vt2", name=f"vt2_{j}")
                def qkv_group(g):
                    ps = qkvps.tile([128, 512], F32, tag="ps", name=f"ps_{j}_{g}")
                    for k in range(KCH):
                        nc.tensor.matmul(
                            ps[:], wsb[:, g, k, :], xt_j[:, k, :],
                            start=(k == 0), stop=(k == KCH - 1),
                        )
                    # bias-add + bf16 cast on the scalar engine
                    bcol = bias_sb[:, g: g + 1]
                    if g == 0:
                        nc.scalar.activation(QT_pair[:, jsl], ps[:], AF.Identity, bias=bcol)
                    elif g == 1:
                        nc.scalar.activation(KT_pair[:, jsl], ps[:], AF.Identity, bias=bcol)
                    elif g == 2:
                        nc.scalar.activation(vt_p[:], ps[:], AF.Identity, bias=bcol)
                    elif g == 3:
                        # [Q_h2 | K_h2] packed in one group
                        nc.scalar.activation(QT_h2[0:64, jsl], ps[0:64, :], AF.Identity,
                                             bias=bias_sb[0:64, 3:4])
                        nc.scalar.activation(KT_h2[64:128, jsl], ps[64:128, :], AF.Identity,
                                             bias=bias_sb[64:128, 3:4])
                    else:  # g == 4: V_h2, rows 0-63 only
                        nc.scalar.activation(vt_2[:], ps[0:64, :], AF.Identity,
                                             bias=bias_sb[0:64, 4:5])

                # V groups first so the V^T->V transpose chain (ACT copy ->
                # xbar DMA -> DVE copy) finishes before the attention phase
                # needs the (whole-tile-tracked) V_aug tiles.
                qkv_group(2)
                qkv_group(4)
                # V^T -> V natural via xbar transpose (contiguous staging; the
                # xbar mis-writes strided out APs on HW) then strided DVE copy
                half, c0 = divmod(4 * j, NCHUNK // 2)
                for h, src_ap in ((0, vt_p[0:64, :]), (1, vt_p[64:128, :]), (2, vt_2[:])):
                    vst = vstpool.tile([128, 4, 64], BF16, tag="vst", name=f"vst_{j}_{h}")
                    nc.sync.dma_start_transpose(vst[:], src_ap)
                    va3 = V_aug[h][half][:].rearrange("p (c m) -> p c m", m=65)
                    nc.vector.tensor_copy(va3[:, c0:c0 + 4, 0:64], vst[:])
                qkv_group(0)
                qkv_group(1)
                qkv_group(3)

                # duplicate h2 Q/K into the other partition half for the
                # even/odd paired score matmuls
                nc.gpsimd.dma_start(QT_h2[64:128, jsl], QT_h2[0:64, jsl])
                nc.gpsimd.dma_start(KT_h2[0:64, jsl], KT_h2[64:128, jsl])

            for j in range(NSC):
                qkv_chunk(j)

        # ---------------- stage C: attention ----------------
        with ExitStack() as cctx:
            spool = cctx.enter_context(tc.tile_pool(name="s", bufs=3, space="PSUM"))
            opool = cctx.enter_context(tc.tile_pool(name="o", bufs=1, space="PSUM"))
            papool = cctx.enter_context(tc.tile_pool(name="pa", bufs=2))
            pipool = cctx.enter_context(tc.tile_pool(name="pi", bufs=2))
            osb_pool = cctx.enter_context(tc.tile_pool(name="osb", bufs=4))
            bcsb = cctx.enter_context(tc.tile_pool(name="bcs", bufs=2))
            dpool = cctx.enter_context(tc.tile_pool(name="dd", bufs=2))
            rpool = cctx.enter_context(tc.tile_pool(name="rr", bufs=2))
            rdpool = cctx.enter_context(tc.tile_pool(name="rd", bufs=2, space="DRAM"))
            ysb_pool = cctx.enter_context(tc.tile_pool(name="ysb", bufs=3))

            def softmax_p(s2, use_act, name):
                """exp(SCALE * s2) -> bf16 [128, 1024]; ACT or DVE variant."""
                if use_act:
                    p2 = papool.tile([128, 1024], BF16, tag="pa", name=f"pa_{name}")
                    nc.scalar.activation(p2[:], s2[:], AF.Exp, scale=SCALE)
                    return p2[:]
                pi = pipool.tile([128, 1024], I16, tag="pi", name=f"pi_{name}")
                nc.vector.tensor_scalar(pi[:], s2[:], EXP_C1, EXP_C2, ALU.mult, ALU.add)
                return pi[:].bitcast(BF16)

            def proj_subtile(pj, t4):
                # output projection of one 128-row q-subtile; borrows an s slot
                t = 4 * pj + t4
                tsl = bass.ts(t, 128)
                ysb = ysb_pool.tile([128, D], F32, tag="ysb", name=f"ysb_{t}")
                for half in range(2):
                    hsl = bass.ts(half, 384)
                    yp = opool.tile([128, 384], F32, tag="o1", name=f"yp_{t}_{half}")
                    nc.tensor.matmul(yp[:], O_pair[:, tsl], wpp_sb[:, hsl],
                                     start=True, stop=False)
                    nc.tensor.matmul(yp[:], O_h2z[:, tsl], wp2z_sb[:, hsl],
                                     start=False, stop=True)
                    nc.scalar.copy(ysb[:, hsl], yp[:])
                nc.sync.dma_start(y[128 * t: 128 * (t + 1), :], ysb[:])

            def normalize(jq, osb_group, h0):
                """Batched denominator reciprocal + DMA broadcast + scale for
                heads h0..h0+len(osb_group)-1 of query tile jq."""
                qsl = bass.ts(jq, 512)
                nh = len(osb_group)
                dd = dpool.tile([nh, 512], F32, tag="d3", name=f"d3_{jq}_{h0}")
                for i, o_sb in enumerate(osb_group):
                    nc.sync.dma_start(dd[i:i + 1, :], o_sb[64:65, :])
                rr = rpool.tile([nh, 512], F32, tag="r3", name=f"r3_{jq}_{h0}")
                nc.vector.reciprocal_approx_fast(rr[:], dd[:])
                rdd = rdpool.tile([nh, 512], F32, tag="rd3", name=f"rd3_{jq}_{h0}")
                nc.gpsimd.dma_start(rdd[:], rr[:])
                for i, o_sb in enumerate(osb_group):
                    h = h0 + i
                    bcs = bcsb.tile([64, 512], F32, tag="bcs", name=f"bcs_{jq}_{h}")
                    nc.gpsimd.dma_start(bcs[:], rdd[i:i + 1, :].to_broadcast([64, 512]))
                    if h < 2:
                        dest = O_pair[64 * h: 64 * (h + 1), qsl]
                    else:
                        dest = O_h2z[0:64, qsl]
                    nc.vector.tensor_mul(dest, o_sb[0:64, :], bcs[:])

            pending = []
            for jq in range(NQT):
                qsl = bass.ts(jq, 512)
                osbs = []

                def mk_s2_pair(c):
                    ksl = bass.ts(c, 128)
                    s2 = spool.tile([128, 1024], F32, tag="s2", name=f"s2_{jq}_{c}")
                    nc.tensor.matmul(s2[:, 0:512], KT_pair[0:64, ksl], QT_pair[0:64, qsl],
                                     start=True, stop=True)
                    nc.tensor.matmul(s2[:, 512:1024], KT_pair[64:128, ksl], QT_pair[64:128, qsl],
                                     start=True, stop=True)
                    return s2

                def mk_s2_h2(cc):
                    s2 = spool.tile([128, 1024], F32, tag="s2", name=f"s2h_{jq}_{cc}")
                    nc.tensor.matmul(s2[:, 0:512], KT_h2[0:64, bass.ts(2 * cc, 128)], QT_h2[0:64, qsl],
                                     start=True, stop=True)
                    nc.tensor.matmul(s2[:, 512:1024], KT_h2[64:128, bass.ts(2 * cc + 1, 128)], QT_h2[64:128, qsl],
                                     start=True, stop=True)
                    return s2

                # heads h0/h1: same key chunk in the two PE row groups.
                # Software-pipelined emit order: the next chunk's (independent)
                # score matmuls are queued BEFORE this chunk's PV matmuls so
                # the in-order PE never head-blocks on the exp result.
                o0 = opool.tile([65, 512], F32, tag="o0")
                o1 = opool.tile([65, 512], F32, tag="o1")
                sA = mk_s2_pair(0)
                sB = mk_s2_pair(1)
                for c in range(NCHUNK):
                    pv = softmax_p(sA, use_act=(c % 2 == 0) or (jq == 0 and c < 8),
                                   name=f"{jq}_{c}")
                    sN = mk_s2_pair(c + 2) if c + 2 < NCHUNK else None
                    nc.tensor.matmul(o0[:], va_chunk(0, c), pv[:, 0:512],
                                     start=(c == 0), stop=(c == NCHUNK - 1))
                    nc.tensor.matmul(o1[:], va_chunk(1, c), pv[:, 512:1024],
                                     start=(c == 0), stop=(c == NCHUNK - 1))
                    sA, sB = sB, sN

                # head h2: even/odd key chunks in the two row groups
                sA = mk_s2_h2(0)
                # free the o0/o1 PSUM banks quickly; normalization is deferred
                for h, o_ps in ((0, o0), (1, o1)):
                    o_sb = osb_pool.tile([65, 512], F32, tag="osb", name=f"osb_{jq}_{h}")
                    nc.scalar.copy(o_sb[:], o_ps[:])
                    osbs.append(o_sb)
                # normalize h0/h1 now -- their recip/broadcast DMA chain hides
                # under the h2 loop instead of adding to the per-jq tail
                normalize(jq, osbs[0:2], 0)
                o2 = opool.tile([65, 512], F32, tag="o0")
                sB = mk_s2_h2(1)
                for cc in range(NCHUNK // 2):
                    ce, co = 2 * cc, 2 * cc + 1
                    if pending and pending[0] <= jq - 1:
                        if cc in (2, 6, 10, 14):
                            pj = pending[0]
                            proj_subtile(pj, {2: 0, 6: 1, 10: 2, 14: 3}[cc])
                            if cc == 14:
                                pending.pop(0)
                    pv = softmax_p(sA, use_act=(cc % 2 == 0), name=f"h2_{jq}_{cc}")
                    sN = mk_s2_h2(cc + 2) if cc + 2 < NCHUNK // 2 else None
                    nc.tensor.matmul(o2[:], va_chunk(2, ce), pv[:, 0:512],
                                     start=(cc == 0), stop=False)
                    nc.tensor.matmul(o2[:], va_chunk(2, co), pv[:, 512:1024],
                                     start=False, stop=(cc == NCHUNK // 2 - 1))
                    sA, sB = sB, sN
                o_sb = osb_pool.tile([65, 512], F32, tag="osb", name=f"osb_{jq}_2")
                nc.scalar.copy(o_sb[:], o2[:])
                osbs.append(o_sb)
                normalize(jq, osbs[2:3], 2)
                pending.append(jq)

            for pj in pending:
                for t4 in range(4):
                    proj_subtile(pj, t4)

    nc.compile()
    return nc


_PROGRAM = None


def _get_program():
    global _PROGRAM
    if _PROGRAM is None:
        _PROGRAM = build_program()
    return _PROGRAM


def make_core_inputs(x, W_qkv, b_qkv, W_proj):
    """Per-core input dicts implementing the (batch, head-group) sharding."""
    x = np.ascontiguousarray(np.asarray(x, np.float32))
    W_qkv = np.asarray(W_qkv, np.float32)
    b_qkv = np.asarray(b_qkv, np.float32)
    W_proj = np.asarray(W_proj, np.float32)
    bf = ml_dtypes.bfloat16

    # xt[b][j, p, k, m] = x[b, 512j + m, 128k + p]
    xts = []
    for b in range(B):
        xb = x[b].astype(bf)                               # [N, D]
        xt = xb.reshape(NSC, 512, KCH, 128)                # [j, m, k, p]
        xt = np.ascontiguousarray(xt.transpose(0, 3, 2, 1))  # [j, p, k, m]
        xts.append(xt)

    ins = []
    for c in range(NC):
        b, g = divmod(c, 4)
        col = 192 * g
        wgq = np.zeros((5, D, 128), np.float32)
        bias = np.zeros((128, 5), np.float32)
        for i, off in enumerate((0, D, 2 * D)):  # q, k, v pair-head blocks
            wgq[i] = W_qkv[:, off + col: off + col + 128]
            bias[:, i] = b_qkv[off + col: off + col + 128]
        q2 = W_qkv[:, col + 128: col + 192]
        k2 = W_qkv[:, D + col + 128: D + col + 192]
        v2 = W_qkv[:, 2 * D + col + 128: 2 * D + col + 192]
        wgq[3] = np.concatenate([q2, k2], axis=1)
        wgq[4, :, 0:64] = v2
        bias[0:64, 3] = b_qkv[col + 128: col + 192]
        bias[64:128, 3] = b_qkv[D + col + 128: D + col + 192]
        bias[0:64, 4] = b_qkv[2 * D + col + 128: 2 * D + col + 192]

        wp2z = np.zeros((128, D), np.float32)
        wp2z[0:64, :] = W_proj[col + 128: col + 192, :]

        ins.append({
            "xt": xts[b],
            "wg": wgq.astype(bf),
            "bias": bias,
            "wpp": W_proj[col: col + 128, :].astype(bf),
            "wp2z": wp2z.astype(bf),
        })
    return ins


def gather_output(results, b_proj):
    b_proj = np.asarray(b_proj, np.float32)
    outs = []
    for b in range(B):
        acc = results[4 * b]["y"].astype(np.float32).copy()
        for c in range(4 * b + 1, 4 * b + 4):
            acc += results[c]["y"]
        outs.append(acc + b_proj)
    return np.stack(outs).astype(np.float32)


def kernel(x, W_qkv, b_qkv, W_proj, b_proj):
    ins = make_core_inputs(x, W_qkv, b_qkv, W_proj)
    prog = _get_program()
    res = run_bass_kernel_spmd(prog, ins, core_ids=list(range(NC)))
    return gather_output(res.results, b_proj)


# revision 23
# speedup vs baseline: 1.0622x; 1.0103x over previous
"""Multi-head attention (B=2, N=4096, D=768, H=12) on 8 Trainium2 NeuronCores.

Sharding: core c handles batch b = c//4 and heads [3g, 3g+1, 3g+2] with
g = c%4 (data parallel on B, head parallel on H). Each core computes its
heads' Q/K/V from x[b], runs softmax attention, and produces the partial
output projection for its head block; the host sums the 4 partials per
batch (row-parallel unshard) and adds b_proj.

v2 changes vs the 674us baseline (which was exp/ACT-throughput bound at
~1205ns per key-chunk with the PE at ~905ns):
  - x^T is pre-transposed and pre-cast to bf16 on the host, removing the
    in-kernel fp32 x load + DVE cast + xbar transpose (~40us of startup
    latency and DMA/DVE pressure).
  - softmax exp alternates between the ACT engine (native Exp) and the
    DVE (Schraudolph bit-trick: p = bitcast_bf16(int16(s*c1 + c2)),
    one tensor_scalar op), halving the softmax-stage cost so the
    attention phase runs at the PE's ~905ns/chunk cadence.
  - softmax denominator reciprocal via reciprocal_approx_fast on a
    [3, 512] batched tile (the baseline's [1,512] nc.vector.reciprocal
    ran single-lane at 8 cyc/elem: 95us -> ~7us).
  - QKV projection drops the duplicated-h2 matmul group (5 groups
    instead of 6); the h2 Q/K partition-duplicates for the even/odd
    score pairing are made with two small SBUF->SBUF DMAs per chunk.
  - O_h2 is stored zero-padded to 128 partitions so the projection
    matmul runs with a 128-row stationary (FWL-eligible).
  - proj PSUM->SBUF copies moved from DVE to the scalar engine.
"""

import numpy as np
import ml_dtypes
from contextlib import ExitStack

import concourse.bass as bass
from concourse import bacc
import concourse.tile as tile
import concourse.mybir as mybir
from concourse.bass_utils import run_bass_kernel_spmd

F32 = mybir.dt.float32
BF16 = mybir.dt.bfloat16
I16 = mybir.dt.int16
AF = mybir.ActivationFunctionType
ALU = mybir.AluOpType

B, N, D, H, HD = 2, 4096, 768, 12, 64
SCALE = HD ** -0.5
NC = 8
NCHUNK = N // 128          # 32 key chunks of 128
NQT = N // 512             # 8 query tiles of 512
NSC = N // 512             # 8 seq chunks of 512 (QKV stage)
KCH = D // 128             # 6 contraction chunks

# Schraudolph exp for the DVE half of the softmax:
#   p = bitcast_bf16(int16(s * EXP_C1 + EXP_C2)) ~= exp(s * SCALE)
# sigma = -5.25 balances the max relative error (~3.3%) for either
# truncating or round-to-nearest float->int conversion.
EXP_C1 = float(SCALE * np.log2(np.e) * 128.0)
EXP_C2 = 16256.0 - 5.25


def build_program():
    nc = bacc.Bacc("TRN2", target_bir_lowering=False, debug=False)

    # xt[j, p, k, m] = x[512j + m, 128k + p] in bf16 (host pre-transposed)
    xt = nc.dram_tensor("xt", [NSC, 128, KCH, 512], BF16, kind="ExternalInput").ap()
    wg = nc.dram_tensor("wg", [5, D, 128], BF16, kind="ExternalInput").ap()
    bias = nc.dram_tensor("bias", [128, 5], F32, kind="ExternalInput").ap()
    wpp = nc.dram_tensor("wpp", [128, D], BF16, kind="ExternalInput").ap()
    wp2z = nc.dram_tensor("wp2z", [128, D], BF16, kind="ExternalInput").ap()
    y = nc.dram_tensor("y", [N, D], F32, kind="ExternalOutput").ap()

    with tile.TileContext(nc) as tc, ExitStack() as octx:
        const = octx.enter_context(tc.tile_pool(name="const", bufs=1))
        qkpool = octx.enter_context(tc.tile_pool(name="qk", bufs=1))
        vpool = octx.enter_context(tc.tile_pool(name="vaug", bufs=1))
        opool_sb = octx.enter_context(tc.tile_pool(name="onorm", bufs=1))

        bias_sb = const.tile([128, 5], F32)
        wpp_sb = const.tile([128, D], BF16)
        wp2z_sb = const.tile([128, D], BF16)
        nc.sync.dma_start(bias_sb[:], bias)
        nc.sync.dma_start(wpp_sb[:], wpp)
        nc.sync.dma_start(wp2z_sb[:], wp2z)

        # [hd, seq] layouts; pair heads stacked on partitions 0-63 / 64-127;
        # the h2 tensors hold the same head duplicated in both halves.
        QT_pair = qkpool.tile([128, N], BF16)
        KT_pair = qkpool.tile([128, N], BF16)
        QT_h2 = qkpool.tile([128, N], BF16)
        KT_h2 = qkpool.tile([128, N], BF16)

        # V natural [seq, hd] per head, chunked [128, 65] with a ones column.
        # Two half tiles per head (key chunks 0-15 / 16-31) so the attention
        # phase's whole-tile dependency doesn't wait for the last QKV chunk.
        V_aug = [
            [vpool.tile([128, (NCHUNK // 2) * 65], BF16, tag=f"vaug{h}_{s}",
                        name=f"vaug{h}_{s}") for s in range(2)]
            for h in range(3)
        ]

        def va_chunk(h, c):
            """AP of V_aug chunk c for head h: [128, 65]."""
            half, cc = divmod(c, NCHUNK // 2)
            return V_aug[h][half][:, 65 * cc: 65 * cc + 65]

        for h in range(3):
            for s in range(2):
                va3 = V_aug[h][s][:].rearrange("p (c m) -> p c m", m=65)
                nc.vector.memset(va3[:, :, 64], 1.0)

        # O^T (normalized) [feat, seq]: pair heads stacked; h2 zero-padded to
        # 128 partitions (rows 64-127 = 0) so proj matmuls get a 128-row
        # stationary (FWL).
        O_pair = opool_sb.tile([128, N], BF16)
        O_h2z = opool_sb.tile([128, N], BF16)
        nc.vector.memset(O_h2z[64:128, :], 0.0)

        ones64 = const.tile([1, 64], F32)
        nc.vector.memset(ones64[:], 1.0)

        # preload the exp activation table while the PE does QKV
        warm = const.tile([1, 1], F32)
        nc.vector.memset(warm[:], 0.0)
        nc.scalar.activation(warm[:], warm[:], AF.Exp)

        # ------------- stage A: QKV projection from pre-transposed x -------------
        with ExitStack() as bctx:
            wpool = bctx.enter_context(tc.tile_pool(name="wqkv", bufs=1))
            xtpool = bctx.enter_context(tc.tile_pool(name="xt", bufs=3))
            vtpool = bctx.enter_context(tc.tile_pool(name="vt", bufs=3))
            vstpool = bctx.enter_context(tc.tile_pool(name="vst", bufs=4))
            qkvps = bctx.enter_context(tc.tile_pool(name="qkv", bufs=4, space="PSUM"))

            wsb = wpool.tile([128, 5, KCH, 128], BF16)
            for g in (2, 4, 0, 1, 3):  # one DMA per group, in use order
                nc.sync.dma_start(wsb[:, g], wg[g].rearrange("(c p) m -> p c m", p=128))

            def qkv_chunk(j):
                jsl = bass.ts(j, 512)
                xt_j = xtpool.tile([128, KCH, 512], BF16, tag="xt", name=f"xt_{j}")
                nc.sync.dma_start(xt_j[:], xt[j])
                vt_p = vtpool.tile([128, 512], BF16, tag="vtp", name=f"vtp_{j}")
                vt_2 = vtpool.tile([64, 512], BF16, tag="vt2", name=f"vt2_{j}")
                def qkv_group(g):
                    ps = qkvps.tile([128, 512], F32, tag="ps", name=f"ps_{j}_{g}")
                    for k in range(KCH):
                        nc.tensor.matmul(
                            ps[:], wsb[:, g, k, :], xt_j[:, k, :],
                            start=(k == 0), stop=(k == KCH - 1),
                        )
                    # bias-add + bf16 cast on the scalar engine
                    bcol = bias_sb[:, g: g + 1]
                    if g == 0:
                        nc.scalar.activation(QT_pair[:, jsl], ps[:], AF.Identity, bias=bcol)
                    elif g == 1:
                        nc.scalar.activation(KT_pair[:, jsl], ps[:], AF.Identity, bias=bcol)
                    elif g == 2:
                        nc.scalar.activation(vt_p[:], ps[:], AF.Identity, bias=bcol)
                    elif g == 3:
                        # [Q_h2 | K_h2] packed in one group
                        nc.scalar.activation(QT_h2[0:64, jsl], ps[0:64, :], AF.Identity,
                                             bias=bias_sb[0:64, 3:4])
                        nc.scalar.activation(KT_h2[64:128, jsl], ps[64:128, :], AF.Identity,
                                             bias=bias_sb[64:128, 3:4])
                    else:  # g == 4: V_h2, rows 0-63 only
                        nc.scalar.activation(vt_2[:], ps[0:64, :], AF.Identity,
                                             bias=bias_sb[0:64, 4:5])

                # V groups first so the V^T->V transpose chain (ACT copy ->
                # xbar DMA -> DVE copy) finishes before the attention phase
                # needs the (whole-tile-tracked) V_aug tiles.
                qkv_group(2)
                qkv_group(4)
                # V^T -> V natural via xbar transpose (contiguous staging; the
                # xbar mis-writes strided out APs on HW) then strided DVE copy
                half, c0 = divmod(4 * j, NCHUNK // 2)
                for h, src_ap in ((0, vt_p[0:64, :]), (1, vt_p[64:128, :]), (2, vt_2[:])):
                    vst = vstpool.tile([128, 4, 64], BF16, tag="vst", name=f"vst_{j}_{h}")
                    nc.sync.dma_start_transpose(vst[:], src_ap)
                    va3 = V_aug[h][half][:].rearrange("p (c m) -> p c m", m=65)
                    nc.vector.tensor_copy(va3[:, c0:c0 + 4, 0:64], vst[:])
                qkv_group(0)
                qkv_group(1)
                qkv_group(3)

                # duplicate h2 Q/K into the other partition half for the
                # even/odd paired score matmuls
                nc.gpsimd.dma_start(QT_h2[64:128, jsl], QT_h2[0:64, jsl])
                nc.gpsimd.dma_start(KT_h2[0:64, jsl], KT_h2[64:128, jsl])

            for j in range(NSC):
                qkv_chunk(j)

        # ---------------- stage C: attention ----------------
        with ExitStack() as cctx:
            spool = cctx.enter_context(tc.tile_pool(name="s", bufs=3, space="PSUM"))
            opool = cctx.enter_context(tc.tile_pool(name="o", bufs=1, space="PSUM"))
            papool = cctx.enter_context(tc.tile_pool(name="pa", bufs=2))
            pipool = cctx.enter_context(tc.tile_pool(name="pi", bufs=2))
            osb_pool = cctx.enter_context(tc.tile_pool(name="osb", bufs=4))
            bcsb = cctx.enter_context(tc.tile_pool(name="bcs", bufs=2))
            dpool = cctx.enter_context(tc.tile_pool(name="dd", bufs=2))
            rpool = cctx.enter_context(tc.tile_pool(name="rr", bufs=2))
            rdpool = cctx.enter_context(tc.tile_pool(name="rd", bufs=2, space="DRAM"))
            ysb_pool = cctx.enter_context(tc.tile_pool(name="ysb", bufs=3))

            def softmax_p(s2, use_act, name):
                """exp(SCALE * s2) -> bf16 [128, 1024]; ACT or DVE variant."""
                if use_act:
                    p2 = papool.tile([128, 1024], BF16, tag="pa", name=f"pa_{name}")
                    nc.scalar.activation(p2[:], s2[:], AF.Exp, scale=SCALE)
                    return p2[:]
                pi = pipool.tile([128, 1024], I16, tag="pi", name=f"pi_{name}")
                nc.vector.tensor_scalar(pi[:], s2[:], EXP_C1, EXP_C2, ALU.mult, ALU.add)
                return pi[:].bitcast(BF16)

            def proj_subtile(pj, t4, tail=False):
                # output projection of one 128-row q-subtile
                t = 4 * pj + t4
                tsl = bass.ts(t, 128)
                ysb = ysb_pool.tile([128, D], F32, tag="ysb", name=f"ysb_{t}")
                for half in range(2):
                    hsl = bass.ts(half, 384)
                    if tail:
                        yp = spool.tile([128, 384], F32, tag="s2", name=f"yp_{t}_{half}")
                    else:
                        yp = opool.tile([128, 384], F32, tag="o1", name=f"yp_{t}_{half}")
                    nc.tensor.matmul(yp[:], O_pair[:, tsl], wpp_sb[:, hsl],
                                     start=True, stop=False)
                    nc.tensor.matmul(yp[:], O_h2z[:, tsl], wp2z_sb[:, hsl],
                                     start=False, stop=True)
                    nc.scalar.copy(ysb[:, hsl], yp[:])
                nc.sync.dma_start(y[128 * t: 128 * (t + 1), :], ysb[:])

            def normalize(jq, osb_group, h0, use_pe=False):
                """Batched denominator reciprocal + DMA broadcast + scale for
                heads h0..h0+len(osb_group)-1 of query tile jq."""
                qsl = bass.ts(jq, 512)
                nh = len(osb_group)
                dd = dpool.tile([nh, 512], F32, tag="d3", name=f"d3_{jq}_{h0}")
                for i, o_sb in enumerate(osb_group):
                    nc.sync.dma_start(dd[i:i + 1, :], o_sb[64:65, :])
                rr = rpool.tile([nh, 512], F32, tag="r3", name=f"r3_{jq}_{h0}")
                nc.vector.reciprocal_approx_fast(rr[:], dd[:])
                if not use_pe:
                    rdd = rdpool.tile([nh, 512], F32, tag="rd3", name=f"rd3_{jq}_{h0}")
                    nc.gpsimd.dma_start(rdd[:], rr[:])
                for i, o_sb in enumerate(osb_group):
                    h = h0 + i
                    if use_pe:
                        # PE ones-matmul broadcast: skips the DRAM round trip
                        bcs = opool.tile([64, 512], F32, tag="o0", name=f"bcp_{jq}_{h}")
                        nc.tensor.matmul(bcs[:], ones64[0:1, :], rr[i:i + 1, :],
                                         start=True, stop=True)
                    else:
                        bcs = bcsb.tile([64, 512], F32, tag="bcs", name=f"bcs_{jq}_{h}")
                        nc.gpsimd.dma_start(bcs[:], rdd[i:i + 1, :].to_broadcast([64, 512]))
                    if h < 2:
                        dest = O_pair[64 * h: 64 * (h + 1), qsl]
                    else:
                        dest = O_h2z[0:64, qsl]
                    nc.vector.tensor_mul(dest, o_sb[0:64, :], bcs[:])

            pending = []
            for jq in range(NQT):
                qsl = bass.ts(jq, 512)
                osbs = []

                def mk_s2_pair(c):
                    ksl = bass.ts(c, 128)
                    s2 = spool.tile([128, 1024], F32, tag="s2", name=f"s2_{jq}_{c}")
                    nc.tensor.matmul(s2[:, 0:512], KT_pair[0:64, ksl], QT_pair[0:64, qsl],
                                     start=True, stop=True)
                    nc.tensor.matmul(s2[:, 512:1024], KT_pair[64:128, ksl], QT_pair[64:128, qsl],
                                     start=True, stop=True)
                    return s2

                def mk_s2_h2(cc):
                    s2 = spool.tile([128, 1024], F32, tag="s2", name=f"s2h_{jq}_{cc}")
                    nc.tensor.matmul(s2[:, 0:512], KT_h2[0:64, bass.ts(2 * cc, 128)], QT_h2[0:64, qsl],
                                     start=True, stop=True)
                    nc.tensor.matmul(s2[:, 512:1024], KT_h2[64:128, bass.ts(2 * cc + 1, 128)], QT_h2[64:128, qsl],
                                     start=True, stop=True)
                    return s2

                # heads h0/h1: same key chunk in the two PE row groups.
                # Software-pipelined emit order: the next chunk's (independent)
                # score matmuls are queued BEFORE this chunk's PV matmuls so
                # the in-order PE never head-blocks on the exp result.
                o0 = opool.tile([65, 512], F32, tag="o0")
                o1 = opool.tile([65, 512], F32, tag="o1")
                sA = mk_s2_pair(0)
                sB = mk_s2_pair(1)
                for c in range(NCHUNK):
                    pv = softmax_p(sA, use_act=(c % 2 == 0) or (jq == 0 and c < 8),
                                   name=f"{jq}_{c}")
                    sN = mk_s2_pair(c + 2) if c + 2 < NCHUNK else None
                    nc.tensor.matmul(o0[:], va_chunk(0, c), pv[:, 0:512],
                                     start=(c == 0), stop=(c == NCHUNK - 1))
                    nc.tensor.matmul(o1[:], va_chunk(1, c), pv[:, 512:1024],
                                     start=(c == 0), stop=(c == NCHUNK - 1))
                    sA, sB = sB, sN

                # head h2: even/odd key chunks in the two row groups
                sA = mk_s2_h2(0)
                # free the o0/o1 PSUM banks quickly; normalization is deferred
                for h, o_ps in ((0, o0), (1, o1)):
                    o_sb = osb_pool.tile([65, 512], F32, tag="osb", name=f"osb_{jq}_{h}")
                    nc.scalar.copy(o_sb[:], o_ps[:])
                    osbs.append(o_sb)
                # normalize h0/h1 now -- their recip/broadcast DMA chain hides
                # under the h2 loop instead of adding to the per-jq tail
                normalize(jq, osbs[0:2], 0)
                o2 = opool.tile([65, 512], F32, tag="o0")
                sB = mk_s2_h2(1)
                for cc in range(NCHUNK // 2):
                    ce, co = 2 * cc, 2 * cc + 1
                    if pending and pending[0] <= jq - 1:
                        if cc in (2, 6, 10, 14):
                            pj = pending[0]
                            proj_subtile(pj, {2: 0, 6: 1, 10: 2, 14: 3}[cc])
                            if cc == 14:
                                pending.pop(0)
                    pv = softmax_p(sA, use_act=(cc % 2 == 0), name=f"h2_{jq}_{cc}")
                    sN = mk_s2_h2(cc + 2) if cc + 2 < NCHUNK // 2 else None
                    nc.tensor.matmul(o2[:], va_chunk(2, ce), pv[:, 0:512],
                                     start=(cc == 0), stop=False)
                    nc.tensor.matmul(o2[:], va_chunk(2, co), pv[:, 512:1024],
                                     start=False, stop=(cc == NCHUNK // 2 - 1))
                    sA, sB = sB, sN
                o_sb = osb_pool.tile([65, 512], F32, tag="osb", name=f"osb_{jq}_2")
                nc.scalar.copy(o_sb[:], o2[:])
                osbs.append(o_sb)
                normalize(jq, osbs[2:3], 2, use_pe=(jq == NQT - 1))
                pending.append(jq)

            for pj in pending:
                for t4 in range(4):
                    proj_subtile(pj, t4, tail=True)

    nc.compile()
    return nc


_PROGRAM = None


def _get_program():
    global _PROGRAM
    if _PROGRAM is None:
        _PROGRAM = build_program()
    return _PROGRAM


def make_core_inputs(x, W_qkv, b_qkv, W_proj):
    """Per-core input dicts implementing the (batch, head-group) sharding."""
    x = np.ascontiguousarray(np.asarray(x, np.float32))
    W_qkv = np.asarray(W_qkv, np.float32)
    b_qkv = np.asarray(b_qkv, np.float32)
    W_proj = np.asarray(W_proj, np.float32)
    bf = ml_dtypes.bfloat16

    # xt[b][j, p, k, m] = x[b, 512j + m, 128k + p]
    xts = []
    for b in range(B):
        xb = x[b].astype(bf)                               # [N, D]
        xt = xb.reshape(NSC, 512, KCH, 128)                # [j, m, k, p]
        xt = np.ascontiguousarray(xt.transpose(0, 3, 2, 1))  # [j, p, k, m]
        xts.append(xt)

    ins = []
    for c in range(NC):
        b, g = divmod(c, 4)
        col = 192 * g
        wgq = np.zeros((5, D, 128), np.float32)
        bias = np.zeros((128, 5), np.float32)
        for i, off in enumerate((0, D, 2 * D)):  # q, k, v pair-head blocks
            wgq[i] = W_qkv[:, off + col: off + col + 128]
            bias[:, i] = b_qkv[off + col: off + col + 128]
        q2 = W_qkv[:, col + 128: col + 192]
        k2 = W_qkv[:, D + col + 128: D + col + 192]
        v2 = W_qkv[:, 2 * D + col + 128: 2 * D + col + 192]
        wgq[3] = np.concatenate([q2, k2], axis=1)
        wgq[4, :, 0:64] = v2
        bias[0:64, 3] = b_qkv[col + 128: col + 192]
        bias[64:128, 3] = b_qkv[D + col + 128: D + col + 192]
        bias[0:64, 4] = b_qkv[2 * D + col + 128: 2 * D + col + 192]

        wp2z = np.zeros((128, D), np.float32)
        wp2z[0:64, :] = W_proj[col + 128: col + 192, :]

        ins.append({
            "xt": xts[b],
            "wg": wgq.astype(bf),
            "bias": bias,
            "wpp": W_proj[col: col + 128, :].astype(bf),
            "wp2z": wp2z.astype(bf),
        })
    return ins


def gather_output(results, b_proj):
    b_proj = np.asarray(b_proj, np.float32)
    outs = []
    for b in range(B):
        acc = results[4 * b]["y"].astype(np.float32).copy()
        for c in range(4 * b + 1, 4 * b + 4):
            acc += results[c]["y"]
        outs.append(acc + b_proj)
    return np.stack(outs).astype(np.float32)


def kernel(x, W_qkv, b_qkv, W_proj, b_proj):
    ins = make_core_inputs(x, W_qkv, b_qkv, W_proj)
    prog = _get_program()
    res = run_bass_kernel_spmd(prog, ins, core_ids=list(range(NC)))
    return gather_output(res.results, b_proj)
